# revision 14
# baseline (speedup 1.0000x reference)
"""Domain-specific batchnorm (DSBatchNorm2 2D path) on 8 Trainium2 cores.

Strategy: feature-parallel sharding. Core c owns features [c*128,(c+1)*128).
Each core sees ALL cells for its features, so per-domain mean/var need no
cross-core reduction (counts come from the host). The host sorts cells by
domain and ships each core a transposed shard [128 features, N cells].

Mode "i8" (default): the host quantizes x per-feature to int8 codes
(s_f = rowmax/127), halving input DMA vs fp16. Since normalization is
scale-invariant, the device normalizes the CODES and folds the scale into
the per-domain affine coefficients (a = gamma*32s/sqrt((32s)^2 var_u+eps),
b = beta - a*mean_u, where u = code/32 is the on-chip fp16 value):

  per chunk:    DMA int8 codes -> SBUF
  per run:      ACT Copy(int8->fp16, scale=1/32) + accum_out -> sum(u)
                Q split: ACT Square(fp16)+accum  |  DVE TTR (u*u)+accum
  per domain:   tiny finalize -> a, b   (streams: early domains' outputs
                overlap later input)
  per run:      DVE tensor_scalar (4x mode) u*a+b -> fp16 out -> DMA

Mode "fp16": prior all-fp16 implementation (~117 us).

DMA: 8.4 MB in + 16.8 MB out per core at ~0.3 B/ns -> ~85 us floor.
"""

import os
from contextlib import ExitStack

import numpy as np

import concourse.bass as bass
import concourse.tile as tile
from concourse import bacc, mybir
from concourse.bass_utils import run_bass_kernel_spmd

N_DOMAIN = 8
EPS = 1e-5
NCORES = 8
P = 128  # SBUF partitions = features per core
ALIGN = 8  # domain block alignment (columns)
TOTAL_ALIGN = 512

MODE = os.environ.get("DSBN_MODE", "i8")  # "i8" | "fp16"
CHUNK = int(os.environ.get("DSBN_CHUNK", "4128"))
Q_ACT_FRAC = float(os.environ.get("DSBN_QFRAC", "0.22"))  # Q share on ACT
Q_GP_FRAC = float(os.environ.get("DSBN_QGP", "0.0"))  # Q share on GPSIMD
QDVE = os.environ.get("DSBN_QDVE", "stt")  # "stt" | "ttr" | "chain"
EDGE_CHUNKS = os.environ.get("DSBN_EDGE", "1")  # small chunks at both ends
U_SPLIT = 0.78  # fp16 mode: fraction of sum(x) columns reduced on VectorE
CONV_SCALE = 1.0 / 32.0  # int8 code -> fp16 u = c/32 (exact, pow2)

_cache: dict = {}


class _Plan:
    pass


def _plan(y: np.ndarray, chunk: int) -> _Plan:
    p = _Plan()
    y = np.asarray(y).astype(np.int64).ravel()
    n = y.shape[0]
    p.n = n
    p.counts = np.bincount(y, minlength=N_DOMAIN).astype(np.int64)
    p.order = np.argsort(y, kind="stable")
    blk = np.maximum((p.counts + ALIGN - 1) // ALIGN * ALIGN, ALIGN)
    np1 = int(blk.sum())
    npad = (np1 + TOTAL_ALIGN - 1) // TOTAL_ALIGN * TOTAL_ALIGN
    blk[-1] += npad - np1  # fold tail pad into the last domain's block
    p.npad = npad
    bstart = np.concatenate([[0], np.cumsum(blk)])[:-1]
    cstart = np.concatenate([[0], np.cumsum(p.counts)])[:-1]
    # column (padded position) of each domain-sorted row
    col_idx = np.empty(n, dtype=np.int64)
    for d in range(N_DOMAIN):
        col_idx[cstart[d] : cstart[d] + p.counts[d]] = bstart[d] + np.arange(
            p.counts[d]
        )
    p.col_idx = col_idx
    # chunks: small chunks at both ends (fast first finalize, short tail)
    sizes = []
    rem = npad
    if EDGE_CHUNKS == "1" and npad > 4 * chunk:
        head = [chunk // 4, chunk // 4, chunk // 2]
        tail = [chunk // 2, chunk // 4, chunk // 4]
        mid = rem - sum(head) - sum(tail)
        nmid = max(1, round(mid / chunk))
        base = mid // nmid // ALIGN * ALIGN
        msizes = [base] * nmid
        msizes[0] += mid - base * nmid
        sizes = head + msizes + tail
    else:
        while rem > 0:
            cl = min(chunk, rem)
            sizes.append(cl)
            rem -= cl
    assert sum(sizes) == npad and all(s % ALIGN == 0 for s in sizes)
    chunks = []
    cs = 0
    for cl in sizes:
        chunks.append((cs, cl))
        cs += cl
    p.chunks = chunks
    # runs = intersections of domain blocks with chunks, in column order
    runs = []  # (col_start, col_len, domain, chunk_index)
    dom_runs = [[] for _ in range(N_DOMAIN)]
    for ci, (cs, cl) in enumerate(chunks):
        ce = cs + cl
        for d in range(N_DOMAIN):
            rs = max(cs, int(bstart[d]))
            re_ = min(ce, int(bstart[d] + blk[d]))
            if rs < re_:
                dom_runs[d].append(len(runs))
                runs.append((rs, re_ - rs, d, ci))
    for d in range(N_DOMAIN):
        rr = dom_runs[d]
        assert rr == list(range(rr[0], rr[-1] + 1))
    p.runs = runs
    p.dom_runs = [(rr[0], rr[-1] + 1) for rr in dom_runs]
    return p


def _run_meta(plan):
    nch = len(plan.chunks)
    chunk_runs = [[] for _ in range(nch)]
    dom_nruns = [0] * N_DOMAIN
    run_slot = []  # index of this run within its domain
    for rs, rl, d, ci in plan.runs:
        chunk_runs[ci].append((rs, rl, d))
        run_slot.append(dom_nruns[d])
        dom_nruns[d] += 1
    dom_last_chunk = [
        max(ci for rs, rl, dd, ci in plan.runs if dd == d) for d in range(N_DOMAIN)
    ]
    return nch, chunk_runs, dom_nruns, run_slot, dom_last_chunk


def _build_i8(plan: _Plan):
    f16 = mybir.dt.float16
    f32 = mybir.dt.float32
    i8 = mybir.dt.int8
    A = mybir.AluOpType
    AF = mybir.ActivationFunctionType
    X = mybir.AxisListType.X
    npad = plan.npad
    D = N_DOMAIN
    nch, chunk_runs, dom_nruns, run_slot, dom_last_chunk = _run_meta(plan)
    clmax = max(cl for _, cl in plan.chunks)

    # greedy Q assignment: "a" (ACT Square), "g" (DVE mult + GPSIMD reduce),
    # "v" (DVE STT) keeping running shares near the configured fractions
    q_eng = []
    act_cols = 0
    gp_cols = 0
    tot_cols = 0
    for rs, rl, d, ci in plan.runs:
        tot_cols += rl
        if act_cols < Q_ACT_FRAC * tot_cols:
            q_eng.append("a")
            act_cols += rl
        elif gp_cols < Q_GP_FRAC * tot_cols:
            q_eng.append("g")
            gp_cols += rl
        else:
            q_eng.append("v")

    nc = bacc.Bacc("TRN2", target_bir_lowering=False, debug=False, num_devices=NCORES)
    xt = nc.dram_tensor("xt", [P, npad], i8, kind="ExternalInput").ap()
    cmat = nc.dram_tensor("cmat", [P, 37], f32, kind="ExternalInput").ap()
    outd = nc.dram_tensor("out", [P, npad], f16, kind="ExternalOutput").ap()

    with tile.TileContext(nc) as tc:
        with ExitStack() as ctx:
            const_p = ctx.enter_context(tc.tile_pool(name="const", bufs=1))
            in_p = ctx.enter_context(tc.tile_pool(name="in8", bufs=5))
            cf_p = ctx.enter_context(tc.tile_pool(name="cf", bufs=1))
            scr_p = ctx.enter_context(tc.tile_pool(name="scr", bufs=1))
            st_p = ctx.enter_context(tc.tile_pool(name="st", bufs=1))
            fin_p = ctx.enter_context(tc.tile_pool(name="fin", bufs=1))
            out_p = ctx.enter_context(tc.tile_pool(name="ot", bufs=2))

            cm = const_p.tile([P, 37], f32, tag="cm")
            nc.sync.dma_start(cm[:], cmat)
            gs32_col = cm[:, 32:33]  # gamma * 32 * s_f
            bet_col = cm[:, 33:34]
            eps_col = cm[:, 34:35]
            s32_col = cm[:, 35:36]  # 32 * s_f  (count==1 passthrough)
            s32sq_col = cm[:, 36:37]  # (32 * s_f)^2

            # dummy Sqrt up front: pulls the ACT table load into the DMA ramp
            warm = const_p.tile([P, 1], f32, tag="warm")
            nc.scalar.activation(warm[:], eps_col, AF.Sqrt, bias=eps_col, scale=1.0)

            # per-domain stat partials + coefficient tiles (separate tiles so
            # Tile's dependency tracking stays per-domain -> early domains
            # finalize and stream output while later input is still arriving)
            p1 = [st_p.tile([P, dom_nruns[d]], f32, tag=f"p1_{d}", name=f"p1_{d}") for d in range(D)]
            p2 = [st_p.tile([P, dom_nruns[d]], f32, tag=f"p2_{d}", name=f"p2_{d}") for d in range(D)]
            av = [fin_p.tile([P, 1], f32, tag=f"av_{d}", name=f"av_{d}") for d in range(D)]
            bv = [fin_p.tile([P, 1], f32, tag=f"bv_{d}", name=f"bv_{d}") for d in range(D)]

            def finalize(d):
                c = float(plan.counts[d])
                if c <= 0.0:
                    nc.vector.memset(av[d][:], 0.0)
                    nc.vector.memset(bv[d][:], 0.0)
                    return
                if c <= 1.0:
                    # count==1 -> out = x = (32 s) * u
                    nc.vector.tensor_scalar(av[d][:], s32_col, 1.0, None, A.mult)
                    nc.vector.memset(bv[d][:], 0.0)
                    return
                s1 = fin_p.tile([P, 1], f32, tag=f"s1_{d}")
                nc.vector.tensor_reduce(out=s1[:], in_=p1[d][:], axis=X, op=A.add)
                s2 = fin_p.tile([P, 1], f32, tag=f"s2_{d}")
                nc.vector.tensor_reduce(out=s2[:], in_=p2[d][:], axis=X, op=A.add)
                mneg = fin_p.tile([P, 1], f32, tag=f"mneg_{d}")
                nc.vector.tensor_scalar(mneg[:], s1[:], -1.0 / c, None, A.mult)
                ex2 = fin_p.tile([P, 1], f32, tag=f"ex2_{d}")
                nc.vector.tensor_scalar(ex2[:], s2[:], 1.0 / c, None, A.mult)
                m2 = fin_p.tile([P, 1], f32, tag=f"m2_{d}")
                nc.vector.tensor_mul(m2[:], mneg[:], mneg[:])
                varu = fin_p.tile([P, 1], f32, tag=f"varu_{d}")
                nc.vector.tensor_sub(varu[:], ex2[:], m2[:])
                vars_ = fin_p.tile([P, 1], f32, tag=f"vars_{d}")
                nc.vector.tensor_mul(vars_[:], varu[:], s32sq_col)
                std = fin_p.tile([P, 1], f32, tag=f"std_{d}")
                nc.scalar.activation(std[:], vars_[:], AF.Sqrt, bias=eps_col, scale=1.0)
                rstd = fin_p.tile([P, 1], f32, tag=f"rstd_{d}")
                nc.vector.reciprocal(rstd[:], std[:])
                nc.vector.tensor_scalar(av[d][:], rstd[:], gs32_col, None, A.mult)
                t1 = fin_p.tile([P, 1], f32, tag=f"t1_{d}")
                nc.vector.tensor_mul(t1[:], mneg[:], av[d][:])
                nc.vector.tensor_scalar(bv[d][:], t1[:], bet_col, None, A.add)

            def pass2(ci):
                cs, cl = plan.chunks[ci]
                t = cf[ci]
                ot = out_p.tile([P, clmax], f16, tag="ot")
                for rs, rl, d in chunk_runs[ci]:
                    lo = rs - cs
                    nc.vector.tensor_scalar(
                        out=ot[:, lo : lo + rl],
                        in0=t[:, lo : lo + rl],
                        scalar1=av[d][:, 0:1],
                        scalar2=bv[d][:, 0:1],
                        op0=A.mult,
                        op1=A.add,
                    )
                nc.sync.dma_start(outd[:, cs : cs + cl], ot[:, :cl])

            cf = {}
            ri = 0
            max_fin = -1
            next_p2 = 0
            for ci in range(nch):
                cs, cl = plan.chunks[ci]
                t8 = in_p.tile([P, clmax], i8, tag="in8")
                nc.sync.dma_start(t8[:, :cl], xt[:, cs : cs + cl])
                tf = cf_p.tile([P, cl], f16, tag=f"cf{ci}", name=f"cf{ci}")
                cf[ci] = tf
                # pass 1a: convert + per-run sum(u) via ACT accumulator
                for rs, rl, d in chunk_runs[ci]:
                    lo = rs - cs
                    slot = run_slot[ri]
                    nc.scalar.activation(
                        tf[:, lo : lo + rl],
                        t8[:, lo : lo + rl],
                        AF.Copy,
                        bias=0.0,
                        scale=CONV_SCALE,
                        accum_out=p1[d][:, slot : slot + 1],
                    )
                    ri += 1
                # pass 1b: per-run sum(u^2), split ACT / DVE. Both read the
                # RAW int8 tile (scale folded in) so Q depends only on the
                # input DMA, never on the convert pass.
                rj = ri - len(chunk_runs[ci])
                for rs, rl, d in chunk_runs[ci]:
                    lo = rs - cs
                    slot = run_slot[rj]
                    if q_eng[rj] == "a":
                        sq = scr_p.tile([P, clmax], f16, tag="sqa")
                        nc.scalar.activation(
                            sq[:, :rl],
                            t8[:, lo : lo + rl],
                            AF.Square,
                            bias=0.0,
                            scale=CONV_SCALE,
                            accum_out=p2[d][:, slot : slot + 1],
                        )
                    elif q_eng[rj] == "g":
                        # DVE 2x mult from fp16 codes, GPSIMD sum-reduce
                        sq = scr_p.tile([P, clmax], f16, tag="sqg")
                        nc.vector.tensor_mul(
                            sq[:, :rl], tf[:, lo : lo + rl], tf[:, lo : lo + rl]
                        )
                        nc.gpsimd.tensor_reduce(
                            out=p2[d][:, slot : slot + 1],
                            in_=sq[:, :rl],
                            axis=X,
                            op=A.add,
                        )
                    else:  # STT: out = (x*(s^2)) * x, accum = sum(u^2)
                        sq = scr_p.tile([P, clmax], f16, tag="sqv")
                        nc.vector.scalar_tensor_tensor(
                            out=sq[:, :rl],
                            in0=t8[:, lo : lo + rl],
                            scalar=CONV_SCALE * CONV_SCALE,
                            in1=t8[:, lo : lo + rl],
                            op0=A.mult,
                            op1=A.mult,
                            accum_out=p2[d][:, slot : slot + 1],
                        )
                    rj += 1
                # finalize any domain whose data is now fully in
                for d in range(D):
                    if dom_last_chunk[d] == ci:
                        finalize(d)
                        max_fin = d
                # emit pass2 for chunks whose domains are all finalized
                while next_p2 < nch and chunk_runs[next_p2][-1][2] <= max_fin:
                    pass2(next_p2)
                    next_p2 += 1
            assert next_p2 == nch and ri == len(plan.runs)

    nc.compile()
    return nc


def _build_fp16(plan: _Plan):
    fdt = mybir.dt.float16
    f32 = mybir.dt.float32
    A = mybir.AluOpType
    AF = mybir.ActivationFunctionType
    X = mybir.AxisListType.X
    npad = plan.npad
    D = N_DOMAIN
    nch, chunk_runs, dom_nruns, run_slot, dom_last_chunk = _run_meta(plan)
    clmax = max(cl for _, cl in plan.chunks)

    nc = bacc.Bacc("TRN2", target_bir_lowering=False, debug=False, num_devices=NCORES)
    xt = nc.dram_tensor("xt", [P, npad], fdt, kind="ExternalInput").ap()
    cmat = nc.dram_tensor("cmat", [P, 35], f32, kind="ExternalInput").ap()
    outd = nc.dram_tensor("out", [P, npad], fdt, kind="ExternalOutput").ap()

    with tile.TileContext(nc) as tc:
        with ExitStack() as ctx:
            const_p = ctx.enter_context(tc.tile_pool(name="const", bufs=1))
            xin_p = ctx.enter_context(tc.tile_pool(name="xin", bufs=1))
            scr_p = ctx.enter_context(tc.tile_pool(name="scr", bufs=2))
            st_p = ctx.enter_context(tc.tile_pool(name="st", bufs=1))
            fin_p = ctx.enter_context(tc.tile_pool(name="fin", bufs=1))
            out_p = ctx.enter_context(tc.tile_pool(name="ot", bufs=3))

            cm = const_p.tile([P, 35], f32, tag="cm")
            nc.sync.dma_start(cm[:], cmat)
            gam_col = cm[:, 32:33]
            bet_col = cm[:, 33:34]
            eps_col = cm[:, 34:35]

            warm = const_p.tile([P, 1], f32, tag="warm")
            nc.scalar.activation(warm[:], eps_col, AF.Sqrt, bias=eps_col, scale=1.0)

            p1 = [st_p.tile([P, 2 * dom_nruns[d]], f32, tag=f"p1_{d}", name=f"p1_{d}") for d in range(D)]
            p2 = [st_p.tile([P, dom_nruns[d]], f32, tag=f"p2_{d}", name=f"p2_{d}") for d in range(D)]
            av = [fin_p.tile([P, 1], f32, tag=f"av_{d}", name=f"av_{d}") for d in range(D)]
            bv = [fin_p.tile([P, 1], f32, tag=f"bv_{d}", name=f"bv_{d}") for d in range(D)]

            def finalize(d):
                c = float(plan.counts[d])
                if c <= 1.0:
                    nc.vector.memset(av[d][:], 1.0)
                    nc.vector.memset(bv[d][:], 0.0)
                    return
                s1 = fin_p.tile([P, 1], f32, tag=f"s1_{d}")
                nc.vector.tensor_reduce(out=s1[:], in_=p1[d][:], axis=X, op=A.add)
                s2 = fin_p.tile([P, 1], f32, tag=f"s2_{d}")
                nc.vector.tensor_reduce(out=s2[:], in_=p2[d][:], axis=X, op=A.add)
                mneg = fin_p.tile([P, 1], f32, tag=f"mneg_{d}")
                nc.vector.tensor_scalar(mneg[:], s1[:], -1.0 / c, None, A.mult)
                ex2 = fin_p.tile([P, 1], f32, tag=f"ex2_{d}")
                nc.vector.tensor_scalar(ex2[:], s2[:], 1.0 / c, None, A.mult)
                m2 = fin_p.tile([P, 1], f32, tag=f"m2_{d}")
                nc.vector.tensor_mul(m2[:], mneg[:], mneg[:])
                var = fin_p.tile([P, 1], f32, tag=f"var_{d}")
                nc.vector.tensor_sub(var[:], ex2[:], m2[:])
                std = fin_p.tile([P, 1], f32, tag=f"std_{d}")
                nc.scalar.activation(std[:], var[:], AF.Sqrt, bias=eps_col, scale=1.0)
                rstd = fin_p.tile([P, 1], f32, tag=f"rstd_{d}")
                nc.vector.reciprocal(rstd[:], std[:])
                nc.vector.tensor_scalar(av[d][:], rstd[:], gam_col, None, A.mult)
                t1 = fin_p.tile([P, 1], f32, tag=f"t1_{d}")
                nc.vector.tensor_mul(t1[:], mneg[:], av[d][:])
                nc.vector.tensor_scalar(bv[d][:], t1[:], bet_col, None, A.add)

            def pass2(ci):
                cs, cl = plan.chunks[ci]
                t = xr[ci]
                ot = out_p.tile([P, clmax], fdt, tag="ot")
                for rs, rl, d in chunk_runs[ci]:
                    lo = rs - cs
                    nc.vector.tensor_scalar(
                        out=ot[:, lo : lo + rl],
                        in0=t[:, lo : lo + rl],
                        scalar1=av[d][:, 0:1],
                        scalar2=bv[d][:, 0:1],
                        op0=A.mult,
                        op1=A.add,
                    )
                nc.sync.dma_start(outd[:, cs : cs + cl], ot[:, :cl])

            xr = {}
            ri = 0
            max_fin = -1
            next_p2 = 0
            for ci in range(nch):
                cs, cl = plan.chunks[ci]
                t = xin_p.tile([P, cl], fdt, tag=f"xr{ci}")
                nc.sync.dma_start(t[:], xt[:, cs : cs + cl])
                xr[ci] = t
                for rs, rl, d in chunk_runs[ci]:
                    lo = rs - cs
                    slot = run_slot[ri]
                    ri += 1
                    k = int(round(U_SPLIT * rl / ALIGN)) * ALIGN
                    if rl - k < 64:
                        k = rl
                    elif k < 64:
                        k = 0
                    if k > 0:
                        h = k // 2
                        scra = scr_p.tile([P, clmax // 2], fdt, tag="scra")
                        nc.vector.tensor_add(
                            scra[:, :h], t[:, lo : lo + h], t[:, lo + h : lo + k]
                        )
                        nc.vector.tensor_reduce(
                            out=p1[d][:, 2 * slot : 2 * slot + 1],
                            in_=scra[:, :h],
                            axis=X,
                            op=A.add,
                        )
                    else:
                        nc.vector.memset(p1[d][:, 2 * slot : 2 * slot + 1], 0.0)
                    if k < rl:
                        assert rl - k <= 1024
                        scr1 = scr_p.tile([P, 1024], fdt, tag="scr1")
                        nc.scalar.activation(
                            scr1[:, : rl - k],
                            t[:, lo + k : lo + rl],
                            AF.Copy,
                            accum_out=p1[d][:, 2 * slot + 1 : 2 * slot + 2],
                        )
                    else:
                        nc.vector.memset(p1[d][:, 2 * slot + 1 : 2 * slot + 2], 0.0)
                    scr2 = scr_p.tile([P, clmax], fdt, tag="scr2")
                    nc.scalar.activation(
                        scr2[:, :rl],
                        t[:, lo : lo + rl],
                        AF.Square,
                        accum_out=p2[d][:, slot : slot + 1],
                    )
                for d in range(D):
                    if dom_last_chunk[d] == ci:
                        finalize(d)
                        max_fin = d
                while next_p2 < nch and chunk_runs[next_p2][-1][2] <= max_fin:
                    pass2(next_p2)
                    next_p2 += 1
            assert next_p2 == nch and ri == len(plan.runs)

    nc.compile()
    return nc


def _prepare(x, y, gamma, beta, mode=None):
    mode = mode or MODE
    x = np.asarray(x)
    if x.dtype != np.float32:
        x = x.astype(np.float32)
    yv = np.asarray(y)
    g = np.asarray(gamma, dtype=np.float32).reshape(-1)
    b = np.asarray(beta, dtype=np.float32).reshape(-1)
    n, f = x.shape
    assert f == P * NCORES, f"expected {P * NCORES} features, got {f}"

    key = (mode, CHUNK, Q_ACT_FRAC, Q_GP_FRAC, QDVE, EDGE_CHUNKS, n, f,
           hash(yv.tobytes()))
    if key in _cache:
        nc, plan = _cache[key]
    else:
        plan = _plan(yv, CHUNK)
        nc = _build_i8(plan) if mode == "i8" else _build_fp16(plan)
        _cache.clear()
        _cache[key] = (nc, plan)

    in_maps = []
    if mode == "i8":
        # per-feature symmetric int8 quantization (scale cancels on device)
        s = np.abs(x).max(axis=0) / 127.0  # [f]
        s[s == 0.0] = 1.0
        codes = np.rint(x * (1.0 / s)[None, :])
        np.clip(codes, -127, 127, out=codes)
        codes = codes.astype(np.int8)
        Xp = np.zeros((plan.npad, f), dtype=np.int8)
        Xp[plan.col_idx] = codes[plan.order]
        s32 = (32.0 * s).astype(np.float32)
        for c in range(NCORES):
            sl = slice(c * P, (c + 1) * P)
            xc = np.ascontiguousarray(Xp[:, sl].T)  # [128, npad] int8
            cmat = np.zeros((P, 37), dtype=np.float32)
            cmat[:, 32] = g[sl] * s32[sl]
            cmat[:, 33] = b[sl]
            cmat[:, 34] = EPS
            cmat[:, 35] = s32[sl]
            cmat[:, 36] = s32[sl] * s32[sl]
            in_maps.append({"xt": xc, "cmat": cmat})
    else:
        Xp = np.zeros((plan.npad, f), dtype=np.float32)
        Xp[plan.col_idx] = x[plan.order]
        for c in range(NCORES):
            sl = slice(c * P, (c + 1) * P)
            xc = Xp[:, sl].T.astype(np.float16)
            cmat = np.zeros((P, 35), dtype=np.float32)
            cmat[:, 32] = g[sl]
            cmat[:, 33] = b[sl]
            cmat[:, 34] = EPS
            in_maps.append({"xt": xc, "cmat": cmat})
    return nc, plan, in_maps, n, f


def _finish(results, plan, n, f):
    out = np.empty((n, f), dtype=np.float32)
    for c in range(NCORES):
        oc = results[c]["out"]  # [128, npad] fp16
        out[plan.order, c * P : (c + 1) * P] = oc[:, plan.col_idx].T.astype(np.float32)
    return out


def kernel(x, y, gamma, beta):
    nc, plan, in_maps, n, f = _prepare(x, y, gamma, beta)
    res = run_bass_kernel_spmd(nc, in_maps, list(range(NCORES)))
    return _finish(res.results, plan, n, f)


def run_profiled(x, y, gamma, beta, mode=None):
    """Like kernel() but with NTFF tracing; returns (out, BassKernelResults)."""
    nc, plan, in_maps, n, f = _prepare(x, y, gamma, beta, mode=mode)
    res = run_bass_kernel_spmd(nc, in_maps, list(range(NCORES)), trace=True)
    return _finish(res.results, plan, n, f), res


# revision 15
# speedup vs baseline: 1.0482x; 1.0482x over previous
"""Domain-specific batchnorm (DSBatchNorm2 2D path) on 8 Trainium2 cores.

Strategy: feature-parallel sharding. Core c owns features [c*128,(c+1)*128).
Each core sees ALL cells for its features, so per-domain mean/var need no
cross-core reduction (counts come from the host). The host sorts cells by
domain and ships each core a transposed shard [128 features, N cells].

Mode "i8" (default): the host quantizes x per-feature to int8 codes
(s_f = rowmax/127), halving input DMA vs fp16. Since normalization is
scale-invariant, the device normalizes the CODES and folds the scale into
the per-domain affine coefficients (a = gamma*32s/sqrt((32s)^2 var_u+eps),
b = beta - a*mean_u, where u = code/32 is the on-chip fp16 value):

  per chunk:    DMA int8 codes -> SBUF
  per run:      ACT Copy(int8->fp16, scale=1/32) + accum_out -> sum(u)
                Q split: ACT Square(fp16)+accum  |  DVE TTR (u*u)+accum
  per domain:   tiny finalize -> a, b   (streams: early domains' outputs
                overlap later input)
  per run:      DVE tensor_scalar (4x mode) u*a+b -> fp16 out -> DMA

Mode "fp16": prior all-fp16 implementation (~117 us).

DMA: 8.4 MB in + 16.8 MB out per core at ~0.3 B/ns -> ~85 us floor.
"""

import os
from contextlib import ExitStack

import numpy as np

import concourse.bass as bass
import concourse.tile as tile
from concourse import bacc, mybir
from concourse.bass_utils import run_bass_kernel_spmd

N_DOMAIN = 8
EPS = 1e-5
NCORES = 8
P = 128  # SBUF partitions = features per core
ALIGN = 8  # domain block alignment (columns)
TOTAL_ALIGN = 512

MODE = os.environ.get("DSBN_MODE", "i8")  # "i8" | "fp16"
CHUNK = int(os.environ.get("DSBN_CHUNK", "4128"))
Q_ACT_FRAC = float(os.environ.get("DSBN_QFRAC", "0.22"))  # Q share on ACT
Q_GP_FRAC = float(os.environ.get("DSBN_QGP", "0.0"))  # Q share on GPSIMD
QDVE = os.environ.get("DSBN_QDVE", "stt")  # "stt" | "ttr" | "chain"
EDGE_CHUNKS = os.environ.get("DSBN_EDGE", "1")  # small chunks at both ends
U_SPLIT = 0.78  # fp16 mode: fraction of sum(x) columns reduced on VectorE
CONV_SCALE = 1.0 / 32.0  # int8 code -> fp16 u = c/32 (exact, pow2)

_cache: dict = {}


class _Plan:
    pass


def _plan(y: np.ndarray, chunk: int) -> _Plan:
    p = _Plan()
    y = np.asarray(y).astype(np.int64).ravel()
    n = y.shape[0]
    p.n = n
    p.counts = np.bincount(y, minlength=N_DOMAIN).astype(np.int64)
    p.order = np.argsort(y, kind="stable")
    blk = np.maximum((p.counts + ALIGN - 1) // ALIGN * ALIGN, ALIGN)
    np1 = int(blk.sum())
    npad = (np1 + TOTAL_ALIGN - 1) // TOTAL_ALIGN * TOTAL_ALIGN
    blk[-1] += npad - np1  # fold tail pad into the last domain's block
    p.npad = npad
    bstart = np.concatenate([[0], np.cumsum(blk)])[:-1]
    cstart = np.concatenate([[0], np.cumsum(p.counts)])[:-1]
    # column (padded position) of each domain-sorted row
    col_idx = np.empty(n, dtype=np.int64)
    for d in range(N_DOMAIN):
        col_idx[cstart[d] : cstart[d] + p.counts[d]] = bstart[d] + np.arange(
            p.counts[d]
        )
    p.col_idx = col_idx
    # chunks: small chunks at both ends (fast first finalize, short tail)
    sizes = []
    rem = npad
    if EDGE_CHUNKS == "1" and npad > 4 * chunk:
        head = [chunk // 4, chunk // 4, chunk // 2]
        tail = [chunk // 2, chunk // 4, chunk // 4]
        mid = rem - sum(head) - sum(tail)
        nmid = max(1, round(mid / chunk))
        base = mid // nmid // ALIGN * ALIGN
        msizes = [base] * nmid
        msizes[0] += mid - base * nmid
        sizes = head + msizes + tail
    else:
        while rem > 0:
            cl = min(chunk, rem)
            sizes.append(cl)
            rem -= cl
    assert sum(sizes) == npad and all(s % ALIGN == 0 for s in sizes)
    chunks = []
    cs = 0
    for cl in sizes:
        chunks.append((cs, cl))
        cs += cl
    p.chunks = chunks
    # runs = intersections of domain blocks with chunks, in column order
    runs = []  # (col_start, col_len, domain, chunk_index)
    dom_runs = [[] for _ in range(N_DOMAIN)]
    for ci, (cs, cl) in enumerate(chunks):
        ce = cs + cl
        for d in range(N_DOMAIN):
            rs = max(cs, int(bstart[d]))
            re_ = min(ce, int(bstart[d] + blk[d]))
            if rs < re_:
                dom_runs[d].append(len(runs))
                runs.append((rs, re_ - rs, d, ci))
    for d in range(N_DOMAIN):
        rr = dom_runs[d]
        assert rr == list(range(rr[0], rr[-1] + 1))
    p.runs = runs
    p.dom_runs = [(rr[0], rr[-1] + 1) for rr in dom_runs]
    return p


def _run_meta(plan):
    nch = len(plan.chunks)
    chunk_runs = [[] for _ in range(nch)]
    dom_nruns = [0] * N_DOMAIN
    run_slot = []  # index of this run within its domain
    for rs, rl, d, ci in plan.runs:
        chunk_runs[ci].append((rs, rl, d))
        run_slot.append(dom_nruns[d])
        dom_nruns[d] += 1
    dom_last_chunk = [
        max(ci for rs, rl, dd, ci in plan.runs if dd == d) for d in range(N_DOMAIN)
    ]
    return nch, chunk_runs, dom_nruns, run_slot, dom_last_chunk


def _build_i8(plan: _Plan):
    f16 = mybir.dt.float16
    f32 = mybir.dt.float32
    i8 = mybir.dt.int8
    A = mybir.AluOpType
    AF = mybir.ActivationFunctionType
    X = mybir.AxisListType.X
    npad = plan.npad
    D = N_DOMAIN
    nch, chunk_runs, dom_nruns, run_slot, dom_last_chunk = _run_meta(plan)
    clmax = max(cl for _, cl in plan.chunks)

    # greedy Q assignment: "a" (ACT Square), "g" (DVE mult + GPSIMD reduce),
    # "v" (DVE STT) keeping running shares near the configured fractions
    q_eng = []
    act_cols = 0
    gp_cols = 0
    tot_cols = 0
    for rs, rl, d, ci in plan.runs:
        tot_cols += rl
        if act_cols < Q_ACT_FRAC * tot_cols:
            q_eng.append("a")
            act_cols += rl
        elif gp_cols < Q_GP_FRAC * tot_cols:
            q_eng.append("g")
            gp_cols += rl
        else:
            q_eng.append("v")

    nc = bacc.Bacc("TRN2", target_bir_lowering=False, debug=False, num_devices=NCORES)
    xt = nc.dram_tensor("xt", [P, npad], i8, kind="ExternalInput").ap()
    cmat = nc.dram_tensor("cmat", [P, 37], f32, kind="ExternalInput").ap()
    outd = nc.dram_tensor("out", [P, npad], f16, kind="ExternalOutput").ap()

    with tile.TileContext(nc) as tc:
        with ExitStack() as ctx:
            const_p = ctx.enter_context(tc.tile_pool(name="const", bufs=1))
            in_p = ctx.enter_context(tc.tile_pool(name="in8", bufs=6))
            cf_p = ctx.enter_context(tc.tile_pool(name="cf", bufs=1))
            scr_p = ctx.enter_context(tc.tile_pool(name="scr", bufs=1))
            st_p = ctx.enter_context(tc.tile_pool(name="st", bufs=1))
            fin_p = ctx.enter_context(tc.tile_pool(name="fin", bufs=1))
            out_p = ctx.enter_context(tc.tile_pool(name="ot", bufs=2))

            cm = const_p.tile([P, 37], f32, tag="cm")
            nc.gpsimd.dma_start(cm[:], cmat)
            gs32_col = cm[:, 32:33]  # gamma * 32 * s_f
            bet_col = cm[:, 33:34]
            eps_col = cm[:, 34:35]
            s32_col = cm[:, 35:36]  # 32 * s_f  (count==1 passthrough)
            s32sq_col = cm[:, 36:37]  # (32 * s_f)^2

            # dummy Sqrt up front: pulls the ACT table load into the DMA ramp
            warm = const_p.tile([P, 1], f32, tag="warm")
            nc.scalar.activation(warm[:], eps_col, AF.Sqrt, bias=eps_col, scale=1.0)

            # per-domain stat partials + coefficient tiles (separate tiles so
            # Tile's dependency tracking stays per-domain -> early domains
            # finalize and stream output while later input is still arriving)
            p1 = [st_p.tile([P, dom_nruns[d]], f32, tag=f"p1_{d}", name=f"p1_{d}") for d in range(D)]
            p2 = [st_p.tile([P, dom_nruns[d]], f32, tag=f"p2_{d}", name=f"p2_{d}") for d in range(D)]
            av = [fin_p.tile([P, 1], f32, tag=f"av_{d}", name=f"av_{d}") for d in range(D)]
            bv = [fin_p.tile([P, 1], f32, tag=f"bv_{d}", name=f"bv_{d}") for d in range(D)]

            def finalize(d):
                c = float(plan.counts[d])
                if c <= 0.0:
                    nc.vector.memset(av[d][:], 0.0)
                    nc.vector.memset(bv[d][:], 0.0)
                    return
                if c <= 1.0:
                    # count==1 -> out = x = (32 s) * u
                    nc.vector.tensor_scalar(av[d][:], s32_col, 1.0, None, A.mult)
                    nc.vector.memset(bv[d][:], 0.0)
                    return
                s1 = fin_p.tile([P, 1], f32, tag=f"s1_{d}")
                nc.vector.tensor_reduce(out=s1[:], in_=p1[d][:], axis=X, op=A.add)
                s2 = fin_p.tile([P, 1], f32, tag=f"s2_{d}")
                nc.vector.tensor_reduce(out=s2[:], in_=p2[d][:], axis=X, op=A.add)
                mneg = fin_p.tile([P, 1], f32, tag=f"mneg_{d}")
                nc.vector.tensor_scalar(mneg[:], s1[:], -1.0 / c, None, A.mult)
                ex2 = fin_p.tile([P, 1], f32, tag=f"ex2_{d}")
                nc.vector.tensor_scalar(ex2[:], s2[:], 1.0 / c, None, A.mult)
                m2 = fin_p.tile([P, 1], f32, tag=f"m2_{d}")
                nc.vector.tensor_mul(m2[:], mneg[:], mneg[:])
                varu = fin_p.tile([P, 1], f32, tag=f"varu_{d}")
                nc.vector.tensor_sub(varu[:], ex2[:], m2[:])
                vars_ = fin_p.tile([P, 1], f32, tag=f"vars_{d}")
                nc.vector.tensor_mul(vars_[:], varu[:], s32sq_col)
                std = fin_p.tile([P, 1], f32, tag=f"std_{d}")
                nc.scalar.activation(std[:], vars_[:], AF.Sqrt, bias=eps_col, scale=1.0)
                rstd = fin_p.tile([P, 1], f32, tag=f"rstd_{d}")
                nc.vector.reciprocal(rstd[:], std[:])
                nc.vector.tensor_scalar(av[d][:], rstd[:], gs32_col, None, A.mult)
                t1 = fin_p.tile([P, 1], f32, tag=f"t1_{d}")
                nc.vector.tensor_mul(t1[:], mneg[:], av[d][:])
                nc.vector.tensor_scalar(bv[d][:], t1[:], bet_col, None, A.add)

            def pass2(ci):
                cs, cl = plan.chunks[ci]
                t = cf[ci]
                ot = out_p.tile([P, clmax], f16, tag="ot")
                for rs, rl, d in chunk_runs[ci]:
                    lo = rs - cs
                    nc.vector.tensor_scalar(
                        out=ot[:, lo : lo + rl],
                        in0=t[:, lo : lo + rl],
                        scalar1=av[d][:, 0:1],
                        scalar2=bv[d][:, 0:1],
                        op0=A.mult,
                        op1=A.add,
                    )
                nc.sync.dma_start(outd[:, cs : cs + cl], ot[:, :cl])

            cf = {}
            ri = 0
            max_fin = -1
            next_p2 = 0
            for ci in range(nch):
                cs, cl = plan.chunks[ci]
                t8 = in_p.tile([P, clmax], i8, tag="in8")
                nc.gpsimd.dma_start(t8[:, :cl], xt[:, cs : cs + cl])
                tf = cf_p.tile([P, cl], f16, tag=f"cf{ci}", name=f"cf{ci}")
                cf[ci] = tf
                # pass 1a: convert + per-run sum(u) via ACT accumulator
                for rs, rl, d in chunk_runs[ci]:
                    lo = rs - cs
                    slot = run_slot[ri]
                    nc.scalar.activation(
                        tf[:, lo : lo + rl],
                        t8[:, lo : lo + rl],
                        AF.Copy,
                        bias=0.0,
                        scale=CONV_SCALE,
                        accum_out=p1[d][:, slot : slot + 1],
                    )
                    ri += 1
                # pass 1b: per-run sum(u^2), split ACT / DVE. Both read the
                # RAW int8 tile (scale folded in) so Q depends only on the
                # input DMA, never on the convert pass.
                rj = ri - len(chunk_runs[ci])
                for rs, rl, d in chunk_runs[ci]:
                    lo = rs - cs
                    slot = run_slot[rj]
                    if q_eng[rj] == "a":
                        sq = scr_p.tile([P, clmax], f16, tag="sqa")
                        nc.scalar.activation(
                            sq[:, :rl],
                            t8[:, lo : lo + rl],
                            AF.Square,
                            bias=0.0,
                            scale=CONV_SCALE,
                            accum_out=p2[d][:, slot : slot + 1],
                        )
                    elif q_eng[rj] == "g":
                        # DVE 2x mult from fp16 codes, GPSIMD sum-reduce
                        sq = scr_p.tile([P, clmax], f16, tag="sqg")
                        nc.vector.tensor_mul(
                            sq[:, :rl], tf[:, lo : lo + rl], tf[:, lo : lo + rl]
                        )
                        nc.gpsimd.tensor_reduce(
                            out=p2[d][:, slot : slot + 1],
                            in_=sq[:, :rl],
                            axis=X,
                            op=A.add,
                        )
                    else:  # STT: out = (x*(s^2)) * x, accum = sum(u^2)
                        sq = scr_p.tile([P, clmax], f16, tag="sqv")
                        nc.vector.scalar_tensor_tensor(
                            out=sq[:, :rl],
                            in0=t8[:, lo : lo + rl],
                            scalar=CONV_SCALE * CONV_SCALE,
                            in1=t8[:, lo : lo + rl],
                            op0=A.mult,
                            op1=A.mult,
                            accum_out=p2[d][:, slot : slot + 1],
                        )
                    rj += 1
                # finalize any domain whose data is now fully in
                for d in range(D):
                    if dom_last_chunk[d] == ci:
                        finalize(d)
                        max_fin = d
                # emit pass2 for chunks whose domains are all finalized
                while next_p2 < nch and chunk_runs[next_p2][-1][2] <= max_fin:
                    pass2(next_p2)
                    next_p2 += 1
            assert next_p2 == nch and ri == len(plan.runs)

    nc.compile()
    return nc


def _build_fp16(plan: _Plan):
    fdt = mybir.dt.float16
    f32 = mybir.dt.float32
    A = mybir.AluOpType
    AF = mybir.ActivationFunctionType
    X = mybir.AxisListType.X
    npad = plan.npad
    D = N_DOMAIN
    nch, chunk_runs, dom_nruns, run_slot, dom_last_chunk = _run_meta(plan)
    clmax = max(cl for _, cl in plan.chunks)

    nc = bacc.Bacc("TRN2", target_bir_lowering=False, debug=False, num_devices=NCORES)
    xt = nc.dram_tensor("xt", [P, npad], fdt, kind="ExternalInput").ap()
    cmat = nc.dram_tensor("cmat", [P, 35], f32, kind="ExternalInput").ap()
    outd = nc.dram_tensor("out", [P, npad], fdt, kind="ExternalOutput").ap()

    with tile.TileContext(nc) as tc:
        with ExitStack() as ctx:
            const_p = ctx.enter_context(tc.tile_pool(name="const", bufs=1))
            xin_p = ctx.enter_context(tc.tile_pool(name="xin", bufs=1))
            scr_p = ctx.enter_context(tc.tile_pool(name="scr", bufs=2))
            st_p = ctx.enter_context(tc.tile_pool(name="st", bufs=1))
            fin_p = ctx.enter_context(tc.tile_pool(name="fin", bufs=1))
            out_p = ctx.enter_context(tc.tile_pool(name="ot", bufs=3))

            cm = const_p.tile([P, 35], f32, tag="cm")
            nc.sync.dma_start(cm[:], cmat)
            gam_col = cm[:, 32:33]
            bet_col = cm[:, 33:34]
            eps_col = cm[:, 34:35]

            warm = const_p.tile([P, 1], f32, tag="warm")
            nc.scalar.activation(warm[:], eps_col, AF.Sqrt, bias=eps_col, scale=1.0)

            p1 = [st_p.tile([P, 2 * dom_nruns[d]], f32, tag=f"p1_{d}", name=f"p1_{d}") for d in range(D)]
            p2 = [st_p.tile([P, dom_nruns[d]], f32, tag=f"p2_{d}", name=f"p2_{d}") for d in range(D)]
            av = [fin_p.tile([P, 1], f32, tag=f"av_{d}", name=f"av_{d}") for d in range(D)]
            bv = [fin_p.tile([P, 1], f32, tag=f"bv_{d}", name=f"bv_{d}") for d in range(D)]

            def finalize(d):
                c = float(plan.counts[d])
                if c <= 1.0:
                    nc.vector.memset(av[d][:], 1.0)
                    nc.vector.memset(bv[d][:], 0.0)
                    return
                s1 = fin_p.tile([P, 1], f32, tag=f"s1_{d}")
                nc.vector.tensor_reduce(out=s1[:], in_=p1[d][:], axis=X, op=A.add)
                s2 = fin_p.tile([P, 1], f32, tag=f"s2_{d}")
                nc.vector.tensor_reduce(out=s2[:], in_=p2[d][:], axis=X, op=A.add)
                mneg = fin_p.tile([P, 1], f32, tag=f"mneg_{d}")
                nc.vector.tensor_scalar(mneg[:], s1[:], -1.0 / c, None, A.mult)
                ex2 = fin_p.tile([P, 1], f32, tag=f"ex2_{d}")
                nc.vector.tensor_scalar(ex2[:], s2[:], 1.0 / c, None, A.mult)
                m2 = fin_p.tile([P, 1], f32, tag=f"m2_{d}")
                nc.vector.tensor_mul(m2[:], mneg[:], mneg[:])
                var = fin_p.tile([P, 1], f32, tag=f"var_{d}")
                nc.vector.tensor_sub(var[:], ex2[:], m2[:])
                std = fin_p.tile([P, 1], f32, tag=f"std_{d}")
                nc.scalar.activation(std[:], var[:], AF.Sqrt, bias=eps_col, scale=1.0)
                rstd = fin_p.tile([P, 1], f32, tag=f"rstd_{d}")
                nc.vector.reciprocal(rstd[:], std[:])
                nc.vector.tensor_scalar(av[d][:], rstd[:], gam_col, None, A.mult)
                t1 = fin_p.tile([P, 1], f32, tag=f"t1_{d}")
                nc.vector.tensor_mul(t1[:], mneg[:], av[d][:])
                nc.vector.tensor_scalar(bv[d][:], t1[:], bet_col, None, A.add)

            def pass2(ci):
                cs, cl = plan.chunks[ci]
                t = xr[ci]
                ot = out_p.tile([P, clmax], fdt, tag="ot")
                for rs, rl, d in chunk_runs[ci]:
                    lo = rs - cs
                    nc.vector.tensor_scalar(
                        out=ot[:, lo : lo + rl],
                        in0=t[:, lo : lo + rl],
                        scalar1=av[d][:, 0:1],
                        scalar2=bv[d][:, 0:1],
                        op0=A.mult,
                        op1=A.add,
                    )
                nc.sync.dma_start(outd[:, cs : cs + cl], ot[:, :cl])

            xr = {}
            ri = 0
            max_fin = -1
            next_p2 = 0
            for ci in range(nch):
                cs, cl = plan.chunks[ci]
                t = xin_p.tile([P, cl], fdt, tag=f"xr{ci}")
                nc.sync.dma_start(t[:], xt[:, cs : cs + cl])
                xr[ci] = t
                for rs, rl, d in chunk_runs[ci]:
                    lo = rs - cs
                    slot = run_slot[ri]
                    ri += 1
                    k = int(round(U_SPLIT * rl / ALIGN)) * ALIGN
                    if rl - k < 64:
                        k = rl
                    elif k < 64:
                        k = 0
                    if k > 0:
                        h = k // 2
                        scra = scr_p.tile([P, clmax // 2], fdt, tag="scra")
                        nc.vector.tensor_add(
                            scra[:, :h], t[:, lo : lo + h], t[:, lo + h : lo + k]
                        )
                        nc.vector.tensor_reduce(
                            out=p1[d][:, 2 * slot : 2 * slot + 1],
                            in_=scra[:, :h],
                            axis=X,
                            op=A.add,
                        )
                    else:
                        nc.vector.memset(p1[d][:, 2 * slot : 2 * slot + 1], 0.0)
                    if k < rl:
                        assert rl - k <= 1024
                        scr1 = scr_p.tile([P, 1024], fdt, tag="scr1")
                        nc.scalar.activation(
                            scr1[:, : rl - k],
                            t[:, lo + k : lo + rl],
                            AF.Copy,
                            accum_out=p1[d][:, 2 * slot + 1 : 2 * slot + 2],
                        )
                    else:
                        nc.vector.memset(p1[d][:, 2 * slot + 1 : 2 * slot + 2], 0.0)
                    scr2 = scr_p.tile([P, clmax], fdt, tag="scr2")
                    nc.scalar.activation(
                        scr2[:, :rl],
                        t[:, lo : lo + rl],
                        AF.Square,
                        accum_out=p2[d][:, slot : slot + 1],
                    )
                for d in range(D):
                    if dom_last_chunk[d] == ci:
                        finalize(d)
                        max_fin = d
                while next_p2 < nch and chunk_runs[next_p2][-1][2] <= max_fin:
                    pass2(next_p2)
                    next_p2 += 1
            assert next_p2 == nch and ri == len(plan.runs)

    nc.compile()
    return nc


def _prepare(x, y, gamma, beta, mode=None):
    mode = mode or MODE
    x = np.asarray(x)
    if x.dtype != np.float32:
        x = x.astype(np.float32)
    yv = np.asarray(y)
    g = np.asarray(gamma, dtype=np.float32).reshape(-1)
    b = np.asarray(beta, dtype=np.float32).reshape(-1)
    n, f = x.shape
    assert f == P * NCORES, f"expected {P * NCORES} features, got {f}"

    key = (mode, CHUNK, Q_ACT_FRAC, Q_GP_FRAC, QDVE, EDGE_CHUNKS, n, f,
           hash(yv.tobytes()))
    if key in _cache:
        nc, plan = _cache[key]
    else:
        plan = _plan(yv, CHUNK)
        nc = _build_i8(plan) if mode == "i8" else _build_fp16(plan)
        _cache.clear()
        _cache[key] = (nc, plan)

    in_maps = []
    if mode == "i8":
        # per-feature symmetric int8 quantization (scale cancels on device)
        s = np.abs(x).max(axis=0) / 127.0  # [f]
        s[s == 0.0] = 1.0
        codes = np.rint(x * (1.0 / s)[None, :])
        np.clip(codes, -127, 127, out=codes)
        codes = codes.astype(np.int8)
        Xp = np.zeros((plan.npad, f), dtype=np.int8)
        Xp[plan.col_idx] = codes[plan.order]
        s32 = (32.0 * s).astype(np.float32)
        for c in range(NCORES):
            sl = slice(c * P, (c + 1) * P)
            xc = np.ascontiguousarray(Xp[:, sl].T)  # [128, npad] int8
            cmat = np.zeros((P, 37), dtype=np.float32)
            cmat[:, 32] = g[sl] * s32[sl]
            cmat[:, 33] = b[sl]
            cmat[:, 34] = EPS
            cmat[:, 35] = s32[sl]
            cmat[:, 36] = s32[sl] * s32[sl]
            in_maps.append({"xt": xc, "cmat": cmat})
    else:
        Xp = np.zeros((plan.npad, f), dtype=np.float32)
        Xp[plan.col_idx] = x[plan.order]
        for c in range(NCORES):
            sl = slice(c * P, (c + 1) * P)
            xc = Xp[:, sl].T.astype(np.float16)
            cmat = np.zeros((P, 35), dtype=np.float32)
            cmat[:, 32] = g[sl]
            cmat[:, 33] = b[sl]
            cmat[:, 34] = EPS
            in_maps.append({"xt": xc, "cmat": cmat})
    return nc, plan, in_maps, n, f


def _finish(results, plan, n, f):
    out = np.empty((n, f), dtype=np.float32)
    for c in range(NCORES):
        oc = results[c]["out"]  # [128, npad] fp16
        out[plan.order, c * P : (c + 1) * P] = oc[:, plan.col_idx].T.astype(np.float32)
    return out


def kernel(x, y, gamma, beta):
    nc, plan, in_maps, n, f = _prepare(x, y, gamma, beta)
    res = run_bass_kernel_spmd(nc, in_maps, list(range(NCORES)))
    return _finish(res.results, plan, n, f)


def run_profiled(x, y, gamma, beta, mode=None):
    """Like kernel() but with NTFF tracing; returns (out, BassKernelResults)."""
    nc, plan, in_maps, n, f = _prepare(x, y, gamma, beta, mode=mode)
    res = run_bass_kernel_spmd(nc, in_maps, list(range(NCORES)), trace=True)
    return _finish(res.results, plan, n, f), res
